# revision 9
# baseline (speedup 1.0000x reference)
"""Trainium2 Bass kernel for a 2-layer GCN encoder (PyG GCNConv semantics).

Strategy (8 NeuronCores, SPMD):
  * Nodes are sharded across the 8 cores (6250 nodes/core); edges are
    partitioned by destination shard; W1/W2/b1/b2 replicated.
  * Layer projections run as dense matmuls on the local node shard.
  * The h = x@W tables are exchanged with an AllGather so every core holds
    the full table, then each core aggregates its destination shard:
    messages are fetched with batched gather DMAs (dma_gather) and reduced
    with one-hot "selection matrix" matmuls on the TensorEngine
    (out[f, n] += msg[e, f]^T @ S[e, n], S[e, n] = norm_e * (dst_e == n)).
  * S is built on-chip by the VectorEngine from tiny per-edge (dst, norm)
    planes: S = (iota == dst) * norm in a single fused tensor_scalar op.
  * Gather tables are stored fp16 (accumulation stays fp32 in PSUM).

All preprocessing (degrees, symmetric norms, edge sorting/padding, int16
gather index tables) happens on the host in numpy inside kernel().
"""

import contextlib
import math
import os
import sys

import numpy as np

for _p in ("/opt/trn_rl_repo", "/root/.axon_site/_ro/trn_rl_repo"):
    if os.path.isdir(_p) and _p not in sys.path:
        sys.path.insert(0, _p)

import concourse.bass as bass
import concourse.bacc as bacc
import concourse.mybir as mybir
import concourse.tile as tile
from concourse.bass_utils import run_bass_kernel_spmd
from concourse.library_config import mlp as _mlp_lib

P = 128
CORES = 8
GRP = 4  # destination-node tiles per gather group

F16 = mybir.dt.float16
F32 = mybir.dt.float32
I16 = mybir.dt.int16


def _cdiv(a, b):
    return -(-a // b)


# debug bisect switches (dev only; harmless defaults)
DBG_SKIP_GATHER = bool(int(os.environ.get("GCN_DBG_SKIP_GATHER", "0")))
DBG_LOCAL_AG = bool(int(os.environ.get("GCN_DBG_LOCAL_AG", "0")))
DBG_STOP_AFTER = os.environ.get("GCN_DBG_STOP_AFTER", "")  # "", "A", "B"


class Plan:
    """Static (cross-core identical) schedule + per-core host arrays."""

    def __init__(
        self, n_nodes, edge_src, edge_dst, in_c, hid, out_c, idx_cap=32768
    ):
        assert n_nodes % CORES == 0
        self.n = n_nodes
        self.inc, self.hid, self.outc = in_c, hid, out_c
        self.shard = n_nodes // CORES
        self.tiles = _cdiv(self.shard, P)
        self.shard_pad = self.tiles * P
        self.npad = self.shard_pad * CORES
        # int16 gather indices: low table is rows [0, split), high table is
        # rows [hibase, npad) addressed relative to hibase.
        self.hibase = max(self.npad - idx_cap, 0)
        self.split = min(idx_cap, self.npad) if self.hibase == 0 else (
            (self.hibase + idx_cap) // 2
        )
        assert self.hibase <= self.split <= idx_cap

        deg = np.bincount(edge_dst, minlength=n_nodes).astype(np.float64) + 1.0
        dinv = 1.0 / np.sqrt(deg)

        shard = self.shard
        core_of = edge_dst // shard
        ln = np.zeros((CORES, self.tiles), np.int64)
        hn = np.zeros((CORES, self.tiles), np.int64)
        percore = []
        for c in range(CORES):
            m = core_of == c
            loops = np.arange(c * shard, (c + 1) * shard, dtype=np.int64)
            s = np.concatenate([edge_src[m], loops])
            d = np.concatenate([edge_dst[m], loops])
            w = (dinv[s] * dinv[d]).astype(np.float32)
            gid = (s // shard) * self.shard_pad + (s % shard)
            dloc = d - c * shard
            t = dloc // P
            drel = dloc % P
            islow = gid < self.split
            percore.append((gid, t, drel, w, islow))
            ln[c] = np.bincount(t[islow], minlength=self.tiles)
            hn[c] = np.bincount(t[~islow], minlength=self.tiles)
        self.low_ch = (ln.max(axis=0) + P - 1) // P  # chunks per tile (low)
        self.high_ch = (hn.max(axis=0) + P - 1) // P

        # group schedule
        self.groups = []
        chunk_base = 0
        idxcol_base = 0
        for g0 in range(0, self.tiles, GRP):
            gt = list(range(g0, min(g0 + GRP, self.tiles)))
            off = 0
            tile_chunks = {}
            for t in gt:
                tile_chunks[t] = list(range(off, off + int(self.low_ch[t])))
                off += int(self.low_ch[t])
            nlowch = off
            for t in gt:
                tile_chunks[t] = tile_chunks[t] + list(
                    range(off, off + int(self.high_ch[t]))
                )
                off += int(self.high_ch[t])
            nhighch = off - nlowch
            self.groups.append(
                dict(
                    tiles=gt,
                    nlowch=nlowch,
                    nhighch=nhighch,
                    nch=off,
                    chunk_base=chunk_base,
                    idxcol_base=idxcol_base,
                    tile_chunks=tile_chunks,
                )
            )
            chunk_base += off
            idxcol_base += off * P // 16
        self.tot_chunks = chunk_base
        self.tot_idxcols = idxcol_base
        self.max_group_nch = max(g["nch"] for g in self.groups)
        self.max_tile_ch = int((self.low_ch + self.high_ch).max())

        # per-core flat edge-position arrays in canonical (call-major) order
        self.core_idx = []
        self.core_drel = []
        self.core_nrm = []
        for c in range(CORES):
            gid, t_arr, drel, w, islow = percore[c]
            idx_flat = np.zeros(self.tot_chunks * P, np.int32)
            drel_flat = np.full(self.tot_chunks * P, -1.0, np.float32)
            nrm_flat = np.zeros(self.tot_chunks * P, np.float32)
            for g in self.groups:
                pos = g["chunk_base"] * P
                for t in g["tiles"]:
                    sel = (t_arr == t) & islow
                    k = int(sel.sum())
                    idx_flat[pos : pos + k] = gid[sel]
                    drel_flat[pos : pos + k] = drel[sel]
                    nrm_flat[pos : pos + k] = w[sel]
                    pos += int(self.low_ch[t]) * P
                for t in g["tiles"]:
                    sel = (t_arr == t) & ~islow
                    k = int(sel.sum())
                    idx_flat[pos : pos + k] = gid[sel] - self.hibase
                    drel_flat[pos : pos + k] = drel[sel]
                    nrm_flat[pos : pos + k] = w[sel]
                    pos += int(self.high_ch[t]) * P
            # wrap idx into the dma_gather SBUF layout [128, cols]:
            # per call, idx i lives at [p, i // 16] for p % 16 == i % 16.
            blocks = []
            for g in self.groups:
                a = g["chunk_base"] * P
                for n in (g["nlowch"] * P, g["nhighch"] * P):
                    if n:
                        v = idx_flat[a : a + n].reshape(-1, 16).T
                        blocks.append(np.tile(v, (8, 1)))
                        a += n
            idx_sb = (
                np.concatenate(blocks, axis=1).astype(np.int16)
                if blocks
                else np.zeros((P, 0), np.int16)
            )
            assert idx_sb.shape == (P, self.tot_idxcols)
            self.core_idx.append(np.ascontiguousarray(idx_sb))
            self.core_drel.append(
                np.ascontiguousarray(
                    drel_flat.reshape(self.tot_chunks, P).T.astype(np.float32)
                )
            )
            self.core_nrm.append(
                np.ascontiguousarray(
                    nrm_flat.reshape(self.tot_chunks, P).T.astype(np.float32)
                )
            )


def _build_nc(plan):
    inc, hid, outc = plan.inc, plan.hid, plan.outc
    ncc = inc // P  # contraction chunks for layer-1 projection
    hcc = hid // P  # contraction chunks for layer-2 projection
    tiles = plan.tiles

    nc = bacc.Bacc("TRN2", num_devices=CORES)

    xt_d = nc.dram_tensor("xt", [tiles, P, ncc, P], F16, kind="ExternalInput")
    w1_d = nc.dram_tensor("w1t", [P, ncc, hid], F16, kind="ExternalInput")
    w2_d = nc.dram_tensor("w2t", [P, hcc, outc], F16, kind="ExternalInput")
    b1_d = nc.dram_tensor("b1c", [P, hcc], F32, kind="ExternalInput")
    b2_d = nc.dram_tensor("b2b", [P, outc], F32, kind="ExternalInput")
    io_d = nc.dram_tensor("iota", [P, P], F16, kind="ExternalInput")
    ix_d = nc.dram_tensor("idx", [P, plan.tot_idxcols], I16, kind="ExternalInput")
    dr_d = nc.dram_tensor("dstrel", [P, plan.tot_chunks], F32, kind="ExternalInput")
    nm_d = nc.dram_tensor("nrm", [P, plan.tot_chunks], F32, kind="ExternalInput")
    out_d = nc.dram_tensor("out", [plan.shard_pad, outc], F32, kind="ExternalOutput")

    h1_sh = nc.dram_tensor("h1_shard", [plan.shard_pad, hid], F16)
    h1_f = nc.dram_tensor("h1_full", [plan.npad, hid], F16, addr_space="Shared")
    h2_sh = nc.dram_tensor("h2_shard", [plan.shard_pad, outc], F16)
    h2_f = nc.dram_tensor("h2_full", [plan.npad, outc], F16, addr_space="Shared")

    st = contextlib.ExitStack()
    idx_sb = st.enter_context(nc.sbuf_tensor("idx_sb", [P, plan.tot_idxcols], I16))
    dr_sb = st.enter_context(nc.sbuf_tensor("dr_sb", [P, plan.tot_chunks], F32))
    nm_sb = st.enter_context(nc.sbuf_tensor("nm_sb", [P, plan.tot_chunks], F32))
    io_sb = st.enter_context(nc.sbuf_tensor("io_sb", [P, P], F16))
    w1_sb = st.enter_context(nc.sbuf_tensor("w1_sb", [P, ncc, hid], F16))
    w2_sb = st.enter_context(nc.sbuf_tensor("w2_sb", [P, hcc, outc], F16))
    b1_sb = st.enter_context(nc.sbuf_tensor("b1_sb", [P, hcc], F32))
    b2_sb = st.enter_context(nc.sbuf_tensor("b2_sb", [P, outc], F32))
    cc1 = st.enter_context(nc.semaphore("cc1"))
    cc2 = st.enter_context(nc.semaphore("cc2"))

    nc.gpsimd.load_library(_mlp_lib)

    # one gpsimd register per distinct gather count (to_reg allocates a
    # physical register per call and never frees it — cache them).
    _regs = {}
    for g in plan.groups:
        for n in (g["nlowch"] * P, g["nhighch"] * P):
            if n and n not in _regs:
                _regs[n] = nc.gpsimd.to_reg(n)


    def _emit_gather(out_ap, table, ic0, n, elem):
        if DBG_SKIP_GATHER:
            nch = out_ap.shape[1]
            nc.sync.dma_start(
                out_ap,
                table[0 : P * nch, :].rearrange("(n p) m -> p n m", p=P),
            )
        else:
            nc.gpsimd.dma_gather(
                out_ap,
                table,
                idx_sb[:, ic0 : ic0 + n // 16],
                n,
                _regs[n],
                elem,
                # >64 descriptors per SDMA engine overflows the single-packet
                # ceiling and wedges the DMA rings — use per-row packets.
                single_packet=(n <= 1024),
            )

    # ctx0: resident loads (own context so its exit barrier orders them
    # before every later consumer on every engine).
    with tile.TileContext(nc):
        nc.sync.dma_start(idx_sb[:, :], ix_d[:, :])
        nc.sync.dma_start(dr_sb[:, :], dr_d[:, :])
        nc.sync.dma_start(nm_sb[:, :], nm_d[:, :])
        nc.sync.dma_start(io_sb[:, :], io_d[:, :])
        nc.sync.dma_start(w1_sb[:, :, :], w1_d[:, :, :])
        nc.sync.dma_start(w2_sb[:, :, :], w2_d[:, :, :])
        nc.sync.dma_start(b1_sb[:, :], b1_d[:, :])
        nc.sync.dma_start(b2_sb[:, :], b2_d[:, :])

    # ctx1: layer-1 dense projection of the local shard: h1 = x @ W1
    with tile.TileContext(nc) as tc:
        with (
            tc.tile_pool(name="pA", bufs=3) as pA,
            tc.tile_pool(name="psA", bufs=4, space="PSUM") as psA,
        ):
            for t in range(tiles):
                xa = pA.tile([P, ncc, P], F16, tag="xa")
                nc.sync.dma_start(xa[:, :, :], xt_d[t, :, :, :])
                ps = psA.tile([P, hid], F32, tag="psA")
                for cc in range(ncc):
                    nc.tensor.matmul(
                        ps[:, :],
                        lhsT=xa[:, cc, :],
                        rhs=w1_sb[:, cc, :],
                        start=(cc == 0),
                        stop=(cc == ncc - 1),
                    )
                h1t = pA.tile([P, hid], F16, tag="h1t")
                nc.vector.tensor_copy(h1t[:, :], ps[:, :])
                nc.sync.dma_start(h1_sh[t * P : (t + 1) * P, :], h1t[:, :])

    if DBG_LOCAL_AG:
        nc.gpsimd.dma_start(
            out=h1_f[0 : plan.shard_pad, :], in_=h1_sh[:, :]
        ).then_inc(cc1, 16)
        nc.gpsimd.wait_ge(cc1, 16)
    else:
        nc.gpsimd.collective_compute(
            "AllGather",
            mybir.AluOpType.bypass,
            replica_groups=[list(range(CORES))],
            ins=[h1_sh[:, :]],
            outs=[h1_f[:, :]],
        ).then_inc(cc1)
        nc.gpsimd.wait_ge(cc1, 1)

    # ctx2: layer-1 aggregate + bias + relu, then layer-2 dense projection
    with tile.TileContext(nc) as tc:
        with (
            tc.tile_pool(name="msgB", bufs=2) as msgB,
            tc.tile_pool(name="sB", bufs=plan.max_tile_ch + 4) as sB,
            tc.tile_pool(name="zB", bufs=3) as zB,
            tc.tile_pool(name="h2B", bufs=3) as h2B,
            tc.tile_pool(name="psB", bufs=GRP + 2, space="PSUM") as psB,
            tc.tile_pool(name="psH", bufs=2, space="PSUM") as psH,
        ):
            for g in plan.groups:
                msg = msgB.tile([P, g["nch"], hid], F16, tag="msg")
                ic = g["idxcol_base"]
                if g["nlowch"]:
                    n = g["nlowch"] * P
                    _emit_gather(msg[:, : g["nlowch"], :], h1_f[:, :], ic, n, hid)
                    ic += n // 16
                if g["nhighch"]:
                    n = g["nhighch"] * P
                    _emit_gather(
                        msg[:, g["nlowch"] : g["nch"], :],
                        h1_f[plan.hibase :, :],
                        ic,
                        n,
                        hid,
                    )
                for t in g["tiles"]:
                    chunks = g["tile_chunks"][t]
                    ps = psB.tile([P, hid], F32, tag="psB")
                    Ss = []
                    for s in chunks:
                        pch = g["chunk_base"] + s
                        S = sB.tile([P, P], F16, tag="S")
                        nc.vector.tensor_scalar(
                            out=S[:, :],
                            in0=io_sb[:, :],
                            scalar1=dr_sb[:, pch : pch + 1],
                            scalar2=nm_sb[:, pch : pch + 1],
                            op0=mybir.AluOpType.is_equal,
                            op1=mybir.AluOpType.mult,
                        )
                        Ss.append(S)
                    last = len(chunks) - 1
                    for j, s in enumerate(chunks):
                        nc.tensor.matmul(
                            ps[:, 0:P],
                            lhsT=msg[:, s, 0:P],
                            rhs=Ss[j][:, :],
                            start=(j == 0),
                            stop=(j == last),
                        )
                    for j, s in enumerate(chunks):
                        nc.tensor.matmul(
                            ps[:, P:hid],
                            lhsT=msg[:, s, P:hid],
                            rhs=Ss[j][:, :],
                            start=(j == 0),
                            stop=(j == last),
                        )
                    z1 = zB.tile([P, hid], F16, tag="z1")
                    for h in range(hcc):
                        nc.scalar.activation(
                            z1[:, h * P : (h + 1) * P],
                            ps[:, h * P : (h + 1) * P],
                            mybir.ActivationFunctionType.Relu,
                            bias=b1_sb[:, h : h + 1],
                        )
                    hps = psH.tile([P, outc], F32, tag="hps")
                    for cc in range(hcc):
                        nc.tensor.matmul(
                            hps[:, :],
                            lhsT=z1[:, cc * P : (cc + 1) * P],
                            rhs=w2_sb[:, cc, :],
                            start=(cc == 0),
                            stop=(cc == hcc - 1),
                        )
                    h2t = h2B.tile([P, outc], F16, tag="h2t")
                    nc.vector.tensor_copy(h2t[:, :], hps[:, :])
                    nc.sync.dma_start(h2_sh[t * P : (t + 1) * P, :], h2t[:, :])

    if DBG_LOCAL_AG:
        nc.gpsimd.dma_start(
            out=h2_f[0 : plan.shard_pad, :], in_=h2_sh[:, :]
        ).then_inc(cc2, 16)
        nc.gpsimd.wait_ge(cc2, 16)
    else:
        nc.gpsimd.collective_compute(
            "AllGather",
            mybir.AluOpType.bypass,
            replica_groups=[list(range(CORES))],
            ins=[h2_sh[:, :]],
            outs=[h2_f[:, :]],
        ).then_inc(cc2)
        nc.gpsimd.wait_ge(cc2, 1)

    # ctx3: layer-2 aggregate + bias -> output
    with tile.TileContext(nc) as tc:
        with (
            tc.tile_pool(name="msgC", bufs=2) as msgC,
            tc.tile_pool(name="sC", bufs=plan.max_tile_ch + 4) as sC,
            tc.tile_pool(name="oC", bufs=3) as oC,
            tc.tile_pool(name="psC", bufs=6, space="PSUM") as psC,
        ):
            for g in plan.groups:
                msg = msgC.tile([P, g["nch"], outc], F16, tag="msg2")
                ic = g["idxcol_base"]
                if g["nlowch"]:
                    n = g["nlowch"] * P
                    _emit_gather(msg[:, : g["nlowch"], :], h2_f[:, :], ic, n, outc)
                    ic += n // 16
                if g["nhighch"]:
                    n = g["nhighch"] * P
                    _emit_gather(
                        msg[:, g["nlowch"] : g["nch"], :],
                        h2_f[plan.hibase :, :],
                        ic,
                        n,
                        outc,
                    )
                for t in g["tiles"]:
                    chunks = g["tile_chunks"][t]
                    ps = psC.tile([P, outc], F32, tag="psC")
                    last = len(chunks) - 1
                    for j, s in enumerate(chunks):
                        pch = g["chunk_base"] + s
                        S = sC.tile([P, P], F16, tag="S2")
                        nc.vector.tensor_scalar(
                            out=S[:, :],
                            in0=io_sb[:, :],
                            scalar1=dr_sb[:, pch : pch + 1],
                            scalar2=nm_sb[:, pch : pch + 1],
                            op0=mybir.AluOpType.is_equal,
                            op1=mybir.AluOpType.mult,
                        )
                        nc.tensor.matmul(
                            ps[:, :],
                            lhsT=S[:, :],
                            rhs=msg[:, s, :],
                            start=(j == 0),
                            stop=(j == last),
                        )
                    ob = oC.tile([P, outc], F32, tag="ob")
                    nc.vector.tensor_tensor(
                        out=ob[:, :],
                        in0=ps[:, :],
                        in1=b2_sb[:, :],
                        op=mybir.AluOpType.add,
                    )
                    nc.sync.dma_start(out_d[t * P : (t + 1) * P, :], ob[:, :])

    st.close()
    nc.compile()
    return nc


def _make_in_maps(plan, x, W1, b1, W2, b2):
    inc, hid, outc = plan.inc, plan.hid, plan.outc
    ncc, hcc = inc // P, hid // P
    w1t = np.ascontiguousarray(
        W1.reshape(ncc, P, hid).transpose(1, 0, 2).astype(np.float16)
    )
    w2t = np.ascontiguousarray(
        W2.reshape(hcc, P, outc).transpose(1, 0, 2).astype(np.float16)
    )
    b1c = np.ascontiguousarray(b1.reshape(hcc, P).T.astype(np.float32))
    b2b = np.ascontiguousarray(np.tile(b2.astype(np.float32), (P, 1)))
    iota = np.ascontiguousarray(
        np.tile(np.arange(P, dtype=np.float16), (P, 1))
    )
    in_maps = []
    for c in range(CORES):
        xs = x[c * plan.shard : (c + 1) * plan.shard].astype(np.float32)
        xs = np.pad(xs, ((0, plan.shard_pad - plan.shard), (0, 0)))
        xt = xs.reshape(plan.tiles, P, ncc, P).transpose(0, 3, 2, 1)
        in_maps.append(
            {
                "xt": np.ascontiguousarray(xt.astype(np.float16)),
                "w1t": w1t,
                "w2t": w2t,
                "b1c": b1c,
                "b2b": b2b,
                "iota": iota,
                "idx": plan.core_idx[c],
                "dstrel": plan.core_drel[c],
                "nrm": plan.core_nrm[c],
            }
        )
    return in_maps


_CACHE = {}


def _get_built(x, edge_index, W1, b1, W2, b2):
    n_nodes, in_c = x.shape
    hid = W1.shape[1]
    out_c = W2.shape[1]
    key = (n_nodes, in_c, hid, out_c, hash(edge_index.tobytes()))
    if key not in _CACHE:
        src = np.asarray(edge_index[0], np.int64)
        dst = np.asarray(edge_index[1], np.int64)
        plan = Plan(n_nodes, src, dst, in_c, hid, out_c)
        nc = _build_nc(plan)
        _CACHE[key] = (plan, nc)
    return _CACHE[key]


def run(x, edge_index, W1, b1, W2, b2, trace=False, **spmd_kwargs):
    plan, nc = _get_built(x, edge_index, W1, b1, W2, b2)
    in_maps = _make_in_maps(plan, x, W1, b1, W2, b2)
    res = run_bass_kernel_spmd(
        nc, in_maps, core_ids=list(range(CORES)), trace=trace, **spmd_kwargs
    )
    out = np.concatenate(
        [res.results[c]["out"][: plan.shard] for c in range(CORES)], axis=0
    ).astype(np.float32)
    return out, res


def kernel(**inputs):
    x = np.asarray(inputs["x"], np.float32)
    edge_index = np.asarray(inputs["edge_index"])
    W1 = np.asarray(inputs["W1"], np.float32)
    b1 = np.asarray(inputs["b1"], np.float32)
    W2 = np.asarray(inputs["W2"], np.float32)
    b2 = np.asarray(inputs["b2"], np.float32)
    out, _ = run(x, edge_index, W1, b1, W2, b2)
    return out
